# revision 12
# baseline (speedup 1.0000x reference)
"""Multi-Head Latent Attention (MLA) Trainium2 kernel, 8-way sharded. v2.

Sharding: 8 cores = 2 (batch) x 4 (head groups of 4 heads). Host sums the
4 partial output projections per batch element.

v2 vs v1:
  - bf16 everywhere on the PE (FWL halves LDWEIGHTS; DMA bytes halved).
  - Q path folded on host: Wq = W_D_Q @ W_U_Q[:,hb]  (saves the qc matmul,
    -5.6e9 MACs/core).
  - No DRAM spills: x, q/k/v, c all SBUF-resident for the whole kernel.
  - Loop nests keep each PE stationary operand for 4x512 moving rows.
  - Softmax row-sums on GpSimd (partition reduce) instead of a ones-matmul.
  - PSUM evacuations pinned to the Vector engine (ACT does only exp).

Everything is computed TRANSPOSED (feature dim on partitions): scores come
out as S^T (keys on partitions), so softmax = plain exp, normalization via
GpSimd partition-sum + reciprocal + broadcast.
"""

import sys

sys.path.insert(0, "/opt/trn_rl_repo")

import numpy as np

import concourse.bacc as bacc
import concourse.mybir as mybir
import concourse.tile as tile
from concourse import bass_isa
from concourse.bass_utils import run_bass_kernel_spmd

# Problem dims (hardcoded per contract)
D, NH, DH, DC, DCQ, DHR = 2048, 16, 128, 512, 1536, 64
B, L = 2, 2048
ROPE_THETA = 10000.0

NHG = 4                 # heads per core
DQB = NHG * DH          # 512: per-core base q/k feature dim (also v dim)
DQR = NHG * DHR         # 256: per-core rope feature dim
P = 128
SCALE = DH ** -0.5

F32 = mybir.dt.float32
F32R = mybir.dt.float32r
BF16 = mybir.dt.bfloat16

KD = D // P            # 16 x k-tiles
KC = DC // P           # 4  c k-tiles
NJ = L // 512          # 4  512-token chunks
LK = L // P            # 16 key tiles

_CACHED = {}


def _build():
    nc = bacc.Bacc("TRN2", target_bir_lowering=False, debug=False)

    # ---- DRAM I/O (per-core data; program is SPMD)
    xT = nc.dram_tensor("xT", [D, L], BF16, kind="ExternalInput")
    wq = nc.dram_tensor("wq", [D, DQB], BF16, kind="ExternalInput")
    wqr = nc.dram_tensor("wqr", [D, DQR], BF16, kind="ExternalInput")
    wkv = nc.dram_tensor("wkv", [D, DC], BF16, kind="ExternalInput")
    wuk = nc.dram_tensor("wuk", [DC, DQB], BF16, kind="ExternalInput")
    wkr = nc.dram_tensor("wkr", [D, DQR], BF16, kind="ExternalInput")
    wuv = nc.dram_tensor("wuv", [DC, DQB], BF16, kind="ExternalInput")
    wo = nc.dram_tensor("wo", [DQB, D], BF16, kind="ExternalInput")
    cosr = nc.dram_tensor("cosr", [P, L], F32, kind="ExternalInput")
    sinr = nc.dram_tensor("sinr", [P, L], F32, kind="ExternalInput")
    protT = nc.dram_tensor("protT", [P, P], BF16, kind="ExternalInput")
    out = nc.dram_tensor("out", [L, D], BF16, kind="ExternalOutput")

    with tile.TileContext(nc) as tc:
        with tc.tile_pool(name="persist", bufs=1) as pp, \
             tc.tile_pool(name="cpool", bufs=1) as cpool:

            # projections computed in phase A1/A2, live until phase B
            qbT = [pp.tile([P, L], BF16, name=f"qbT{h}", tag=f"qbT{h}")
                   for h in range(NHG)]
            qrT = [pp.tile([P, L], BF16, name=f"qrT{m}", tag=f"qrT{m}")
                   for m in range(2)]
            krT = [pp.tile([P, L], BF16, name=f"krT{m}", tag=f"krT{m}")
                   for m in range(2)]
            cts = [cpool.tile([P, L], BF16, name=f"c{k}", tag=f"c{k}")
                   for k in range(KC)]

            def proj_blocks(w_t, nk, xs, m0, m1, evac):
                """For each feature block m in [m0,m1): accumulate over nk
                contraction tiles with the stationary weight serving all 4
                512-token chunks, then evacuate via `evac(m, ps_list)`."""
                for m in range(m0, m1):
                    ps = [psA.tile([P, 512], F32, name="ps", tag="ps")
                          for _ in range(NJ)]
                    for k in range(nk):
                        stat = w_t[:, k, m * P:(m + 1) * P]
                        for j in range(NJ):
                            nc.tensor.matmul(
                                ps[j][:], stat, xs[k][:, j * 512:(j + 1) * 512],
                                start=(k == 0), stop=(k == nk - 1))
                    evac(m, ps)

            def rope_evac(dst_tile):
                """Returns evac fn: raw rope block -> rotate+modulate -> dst."""
                def evac(m, ps):
                    raw = ropep.tile([P, L], F32R, name="raw", tag="raw")
                    for j in range(NJ):
                        nc.vector.tensor_copy(raw[:, j * 512:(j + 1) * 512],
                                              ps[j][:])
                    rawb = ropep.tile([P, L], BF16, name="rawb", tag="rawb")
                    for j in range(NJ):
                        nc.vector.tensor_copy(rawb[:, j * 512:(j + 1) * 512],
                                              ps[j][:])
                    for j in range(NJ):
                        sl = slice(j * 512, (j + 1) * 512)
                        rps = psA.tile([P, 512], F32, name="rps", tag="ps")
                        nc.tensor.matmul(rps[:], prot_t[:], rawb[:, sl],
                                         start=True, stop=True)
                        t1 = rtmp.tile([P, 512], F32, name="t1", tag="t1")
                        nc.vector.tensor_mul(t1[:], cos_t[:, sl], raw[:, sl])
                        t2 = rtmp.tile([P, 512], F32, name="t2", tag="t2")
                        nc.vector.tensor_mul(t2[:], sin_t[:, sl], rps[:])
                        nc.vector.tensor_add(dst_tile[:, sl], t1[:], t2[:])
                return evac

            # ================= Phase A: projections =========================
            with tc.tile_pool(name="xp", bufs=1) as xp:
                # rope tables live in the x scope: both die after A2
                prot_t = xp.tile([P, P], BF16, name="prot_t", tag="prot")
                nc.sync.dma_start(out=prot_t[:], in_=protT[:, :])
                cos_t = xp.tile([P, L], F32, name="cos_t", tag="cos")
                nc.sync.dma_start(out=cos_t[:], in_=cosr[:, :])
                sin_t = xp.tile([P, L], F32, name="sin_t", tag="sin")
                nc.sync.dma_start(out=sin_t[:], in_=sinr[:, :])
                xs = []
                for k in range(KD):
                    xt = xp.tile([P, L], BF16, name="xt", tag=f"xt{k}")
                    nc.sync.dma_start(out=xt[:], in_=xT[k * P:(k + 1) * P, :])
                    xs.append(xt)

                # --- A1: q_base + q_rope (folded weights; contraction = x)
                with tc.tile_pool(name="wqp", bufs=1) as wqp, \
                     tc.tile_pool(name="ropep", bufs=1) as ropep, \
                     tc.tile_pool(name="rtmp", bufs=2) as rtmp, \
                     tc.tile_pool(name="psA", bufs=8, space="PSUM") as psA:
                    wq_t = wqp.tile([P, KD, DQB], BF16, name="wq_t", tag="wq")
                    nc.sync.dma_start(
                        out=wq_t[:], in_=wq.rearrange("(k p) j -> p k j", p=P))
                    wqr_t = wqp.tile([P, KD, DQR], BF16, name="wqr_t", tag="wqr")
                    nc.sync.dma_start(
                        out=wqr_t[:], in_=wqr.rearrange("(k p) j -> p k j", p=P))

                    def evac_qb(m, ps):
                        for j in range(NJ):
                            nc.vector.tensor_copy(
                                qbT[m][:, j * 512:(j + 1) * 512], ps[j][:])
                    proj_blocks(wq_t, KD, xs, 0, NHG, evac_qb)
                    for m in range(2):
                        proj_blocks(wqr_t, KD, xs, m, m + 1,
                                    lambda _m, ps: rope_evac(qrT[m])(_m, ps))

                # --- A2: k_rope (+rope) and latent c
                with tc.tile_pool(name="wkp", bufs=1) as wkp, \
                     tc.tile_pool(name="ropep", bufs=1) as ropep, \
                     tc.tile_pool(name="rtmp", bufs=2) as rtmp, \
                     tc.tile_pool(name="psA", bufs=8, space="PSUM") as psA:
                    wkr_t = wkp.tile([P, KD, DQR], BF16, name="wkr_t", tag="wkr")
                    nc.sync.dma_start(
                        out=wkr_t[:], in_=wkr.rearrange("(k p) j -> p k j", p=P))
                    wkv_t = wkp.tile([P, KD, DC], BF16, name="wkv_t", tag="wkv")
                    nc.sync.dma_start(
                        out=wkv_t[:], in_=wkv.rearrange("(k p) j -> p k j", p=P))

                    for m in range(2):
                        proj_blocks(wkr_t, KD, xs, m, m + 1,
                                    lambda _m, ps: rope_evac(krT[m])(_m, ps))

                    def evac_c(m, ps):
                        for j in range(NJ):
                            nc.vector.tensor_copy(
                                cts[m][:, j * 512:(j + 1) * 512], ps[j][:])
                    proj_blocks(wkv_t, KD, xs, 0, KC, evac_c)

            # ---- pools for A3/B/C outputs (opened after x is freed)
            with tc.tile_pool(name="kvp", bufs=1) as kvp, \
                 tc.tile_pool(name="oTp", bufs=1) as oTp, \
                 tc.tile_pool(name="wop", bufs=1) as wop:
              kbT = [kvp.tile([P, L], BF16, name=f"kbT{h}", tag=f"kbT{h}")
                     for h in range(NHG)]
              vts = [kvp.tile([P, DQB], BF16, name=f"v{lt}", tag=f"v{lt}")
                     for lt in range(LK)]
              oT = [oTp.tile([P, L], BF16, name=f"oT{h}", tag=f"oT{h}")
                    for h in range(NHG)]

              # --- A3: k_base and v (contraction = c); x is freed
              with tc.tile_pool(name="wup", bufs=1) as wup, \
                   tc.tile_pool(name="psA", bufs=8, space="PSUM") as psA:
                wuk_t = wup.tile([P, KC, DQB], BF16, name="wuk_t", tag="wuk")
                nc.sync.dma_start(
                    out=wuk_t[:], in_=wuk.rearrange("(k p) j -> p k j", p=P))
                wuv_t = wup.tile([P, KC, DQB], BF16, name="wuv_t", tag="wuv")
                nc.sync.dma_start(
                    out=wuv_t[:], in_=wuv.rearrange("(k p) j -> p k j", p=P))

                def evac_kb(m, ps):
                    for j in range(NJ):
                        nc.vector.tensor_copy(
                            kbT[m][:, j * 512:(j + 1) * 512], ps[j][:])
                proj_blocks(wuk_t, KC, cts, 0, NHG, evac_kb)

                # v natural: stationary = c token-block, moving = W_U_V k-tile
                for lt in range(LK):
                    ps = psA.tile([P, DQB], F32, name="ps_v", tag="ps")
                    for k in range(KC):
                        nc.tensor.matmul(
                            ps[:], cts[k][:, lt * P:(lt + 1) * P], wuv_t[:, k, :],
                            start=(k == 0), stop=(k == KC - 1))
                    nc.vector.tensor_copy(vts[lt][:], ps[:])

                # prefetch W_O for phase C while B runs
                wo_t = wop.tile([P, NHG, D], BF16, name="wo_t", tag="wo")
                nc.sync.dma_start(
                    out=wo_t[:], in_=wo.rearrange("(k p) j -> p k j", p=P))

              # ================= Phase B: attention =========================
              with tc.tile_pool(name="ptp", bufs=1) as ptp, \
                   tc.tile_pool(name="rsp", bufs=1) as rsp, \
                   tc.tile_pool(name="psB", bufs=4, space="PSUM") as psB:
                for h in range(NHG):
                    qr_m, ro = qrT[h // 2], (h % 2) * DHR
                    kr_m = krT[h // 2]
                    ptb = [ptp.tile([P, LK, 512], BF16, name=f"ptb{lq}",
                                    tag=f"ptb{lq}") for lq in range(NJ)]
                    ot = [psB.tile([P, 512], F32, name="ot", tag="ot")
                          for _ in range(NJ)]
                    for lk in range(LK):
                        ksl = slice(lk * P, (lk + 1) * P)
                        for lq in range(NJ):
                            qsl = slice(lq * 512, (lq + 1) * 512)
                            st = psB.tile([P, 512], F32, name="st", tag="st",
                                          bufs=4)
                            nc.tensor.matmul(st[:], kbT[h][:, ksl],
                                             qbT[h][:, qsl],
                                             start=True, stop=False)
                            nc.tensor.matmul(
                                st[:], kr_m[ro:ro + DHR, ksl],
                                qr_m[ro:ro + DHR, qsl],
                                start=False, stop=True)
                            nc.scalar.activation(
                                ptb[lq][:, lk, :], st[:],
                                mybir.ActivationFunctionType.Exp, scale=SCALE)
                        for lq in range(NJ):
                            nc.tensor.matmul(
                                ot[lq][:], vts[lk][:, h * DH:(h + 1) * DH],
                                ptb[lq][:, lk, :],
                                start=(lk == 0), stop=(lk == LK - 1))
                    for lq in range(NJ):
                        # softmax denominator: partition-sum on GpSimd,
                        # then fold the 16 key-tiles on DVE. bf16 store of
                        # the 16 per-tile partials costs ~0.4% on the
                        # denominator (internal accumulation is f32).
                        rsum = rsp.tile([1, LK, 512], BF16, name="rsum",
                                        tag="rsum")
                        with nc.allow_low_precision("rowsum partials bf16"):
                            nc.gpsimd.tensor_reduce(
                                rsum[:], ptb[lq][:],
                                axis=mybir.AxisListType.C,
                                op=mybir.AluOpType.add)
                        rs1 = rsp.tile([1, 512], F32, name="rs1", tag="rs1")
                        nc.vector.tensor_reduce(
                            rs1[:], rsum[:].rearrange("p k j -> p j k"),
                            axis=mybir.AxisListType.X, op=mybir.AluOpType.add)
                        rec1 = rsp.tile([1, 512], F32, name="rec1", tag="rec1")
                        nc.vector.reciprocal(rec1[:], rs1[:])
                        recb = rsp.tile([P, 512], F32, name="recb", tag="recb")
                        nc.gpsimd.partition_broadcast(recb[:], rec1[:],
                                                      channels=P)
                        nc.vector.tensor_mul(
                            oT[h][:, lq * 512:(lq + 1) * 512], ot[lq][:],
                            recb[:])

              # ================= Phase C: output projection =================
              with tc.tile_pool(name="ocp", bufs=4) as ocp, \
                   tc.tile_pool(name="psC", bufs=8, space="PSUM") as psC:
                for mt in range(L // P):
                    msl = slice(mt * P, (mt + 1) * P)
                    ps = [psC.tile([P, 512], F32, name="ps_o", tag="psc")
                          for _ in range(NJ)]
                    for k in range(NHG):
                        stat = oT[k][:, msl]
                        for nt in range(NJ):
                            nc.tensor.matmul(
                                ps[nt][:], stat, wo_t[:, k, nt * 512:(nt + 1) * 512],
                                start=(k == 0), stop=(k == NHG - 1))
                    oc = ocp.tile([P, D], BF16, name="oc", tag="oc")
                    for nt in range(NJ):
                        nc.vector.tensor_copy(oc[:, nt * 512:(nt + 1) * 512],
                                              ps[nt][:])
                    nc.sync.dma_start(out=out[msl, :], in_=oc[:])

    nc.compile()
    return nc


def _rope_tables():
    """cos/sin in transposed, 2-head-replicated layout (128 x L), plus Prot^T."""
    inv_freq = 1.0 / (ROPE_THETA ** (np.arange(0, DHR, 2, dtype=np.float32) / DHR))
    ang = np.arange(L, dtype=np.float32)[:, None] * inv_freq[None, :]  # (L, 32)
    cos64 = np.concatenate([np.cos(ang), np.cos(ang)], axis=1).T  # (64, L)
    sin64 = np.concatenate([np.sin(ang), np.sin(ang)], axis=1).T
    cosr = np.ascontiguousarray(np.tile(cos64, (2, 1)), dtype=np.float32)
    sinr = np.ascontiguousarray(np.tile(sin64, (2, 1)), dtype=np.float32)
    # rot(x) = [-x2, x1] per 64-dim head; block-diag over 2 heads; transposed.
    p64 = np.zeros((DHR, DHR), dtype=np.float32)
    half = DHR // 2
    p64[np.arange(half), np.arange(half) + half] = -1.0
    p64[np.arange(half) + half, np.arange(half)] = 1.0
    p128 = np.zeros((P, P), dtype=np.float32)
    p128[:DHR, :DHR] = p64
    p128[DHR:, DHR:] = p64
    protT = np.ascontiguousarray(p128.T)
    return cosr, sinr, protT


def _bf16():
    return mybir.dt.np(BF16)


def build_in_maps(x, W_D_Q, W_U_Q, W_Q_R, W_D_KV, W_U_K, W_K_R, W_U_V, W_O):
    cosr, sinr, protT = _rope_tables()
    bf = _bf16()
    f = np.float32
    x = np.asarray(x, f)
    W_D_Q = np.asarray(W_D_Q, f)
    # fold the low-rank query compression into single input->head projections
    Wq_full = (W_D_Q @ np.asarray(W_U_Q, f)).astype(bf)     # (D, NH*DH)
    Wqr_full = (W_D_Q @ np.asarray(W_Q_R, f)).astype(bf)    # (D, NH*DHR)
    xTs = [np.ascontiguousarray(x[b].T).astype(bf) for b in range(B)]
    wkv_b = np.ascontiguousarray(np.asarray(W_D_KV, f)).astype(bf)
    in_maps = []
    for c in range(8):
        b, g = c // 4, c % 4
        hb = slice(g * DQB, (g + 1) * DQB)
        hr = slice(g * DQR, (g + 1) * DQR)
        in_maps.append(dict(
            xT=xTs[b],
            wq=np.ascontiguousarray(Wq_full[:, hb]),
            wqr=np.ascontiguousarray(Wqr_full[:, hr]),
            wkv=wkv_b,
            wuk=np.ascontiguousarray(np.asarray(W_U_K, f)[:, hb]).astype(bf),
            wkr=np.ascontiguousarray(np.asarray(W_K_R, f)[:, hr]).astype(bf),
            wuv=np.ascontiguousarray(np.asarray(W_U_V, f)[:, hb]).astype(bf),
            wo=np.ascontiguousarray(np.asarray(W_O, f)[hb, :]).astype(bf),
            cosr=cosr, sinr=sinr,
            protT=protT.astype(bf),
        ))
    return in_maps


def kernel(x, W_D_Q, W_U_Q, W_Q_R, W_D_KV, W_U_K, W_K_R, W_U_V, W_O):
    if "nc" not in _CACHED:
        _CACHED["nc"] = _build()
    nc = _CACHED["nc"]
    in_maps = build_in_maps(x, W_D_Q, W_U_Q, W_Q_R, W_D_KV, W_U_K, W_K_R,
                            W_U_V, W_O)
    res = run_bass_kernel_spmd(nc, in_maps, core_ids=list(range(8)))
    outs = [r["out"].astype(np.float32) for r in res.results]
    full = np.stack(
        [outs[b * 4] + outs[b * 4 + 1] + outs[b * 4 + 2] + outs[b * 4 + 3]
         for b in range(B)]).astype(np.float32)
    return full


# revision 16
# speedup vs baseline: 25.7079x; 25.7079x over previous
"""Multi-Head Latent Attention (MLA) Trainium2 kernel, 8-way sharded. v2.

Sharding: 8 cores = 2 (batch) x 4 (head groups of 4 heads). Host sums the
4 partial output projections per batch element.

v2 vs v1:
  - bf16 everywhere on the PE (FWL halves LDWEIGHTS; DMA bytes halved).
  - Q path folded on host: Wq = W_D_Q @ W_U_Q[:,hb]  (saves the qc matmul,
    -5.6e9 MACs/core).
  - No DRAM spills: x, q/k/v, c all SBUF-resident for the whole kernel.
  - Loop nests keep each PE stationary operand for 4x512 moving rows.
  - Softmax row-sums on GpSimd (partition reduce) instead of a ones-matmul.
  - PSUM evacuations pinned to the Vector engine (ACT does only exp).

Everything is computed TRANSPOSED (feature dim on partitions): scores come
out as S^T (keys on partitions), so softmax = plain exp, normalization via
GpSimd partition-sum + reciprocal + broadcast.
"""

import sys

sys.path.insert(0, "/opt/trn_rl_repo")

import numpy as np

import concourse.bacc as bacc
import concourse.mybir as mybir
import concourse.tile as tile
from concourse import bass_isa
from concourse.bass_utils import run_bass_kernel_spmd

# Problem dims (hardcoded per contract)
D, NH, DH, DC, DCQ, DHR = 2048, 16, 128, 512, 1536, 64
B, L = 2, 2048
ROPE_THETA = 10000.0

NHG = 4                 # heads per core
DQB = NHG * DH          # 512: per-core base q/k feature dim (also v dim)
DQR = NHG * DHR         # 256: per-core rope feature dim
P = 128
SCALE = DH ** -0.5

F32 = mybir.dt.float32
F32R = mybir.dt.float32r
BF16 = mybir.dt.bfloat16

KD = D // P            # 16 x k-tiles
KC = DC // P           # 4  c k-tiles
NJ = L // 512          # 4  512-token chunks
LK = L // P            # 16 key tiles

_CACHED = {}


def _build():
    nc = bacc.Bacc("TRN2", target_bir_lowering=False, debug=False)

    # ---- DRAM I/O (per-core data; program is SPMD)
    xT = nc.dram_tensor("xT", [D, L], BF16, kind="ExternalInput")
    wq = nc.dram_tensor("wq", [D, DQB], BF16, kind="ExternalInput")
    wqr = nc.dram_tensor("wqr", [D, DQR], BF16, kind="ExternalInput")
    wkv = nc.dram_tensor("wkv", [D, DC], BF16, kind="ExternalInput")
    wuk = nc.dram_tensor("wuk", [DC, DQB], BF16, kind="ExternalInput")
    wkr = nc.dram_tensor("wkr", [D, DQR], BF16, kind="ExternalInput")
    wuv = nc.dram_tensor("wuv", [DC, DQB], BF16, kind="ExternalInput")
    wo = nc.dram_tensor("wo", [DQB, D], BF16, kind="ExternalInput")
    cosr = nc.dram_tensor("cosr", [P, L], F32, kind="ExternalInput")
    sinr = nc.dram_tensor("sinr", [P, L], F32, kind="ExternalInput")
    protT = nc.dram_tensor("protT", [P, P], BF16, kind="ExternalInput")
    onesd = nc.dram_tensor("onesd", [P, P], BF16, kind="ExternalInput")
    out = nc.dram_tensor("out", [L, D], BF16, kind="ExternalOutput")

    with tile.TileContext(nc) as tc:
        with tc.tile_pool(name="persist", bufs=1) as pp, \
             tc.tile_pool(name="cpool", bufs=1) as cpool:

            # projections computed in phase A1/A2, live until phase B
            qbT = [pp.tile([P, L], BF16, name=f"qbT{h}", tag=f"qbT{h}")
                   for h in range(NHG)]
            qrT = [pp.tile([P, L], BF16, name=f"qrT{m}", tag=f"qrT{m}")
                   for m in range(2)]
            krT = [pp.tile([P, L], BF16, name=f"krT{m}", tag=f"krT{m}")
                   for m in range(2)]
            cts = [cpool.tile([P, L], BF16, name=f"c{k}", tag=f"c{k}")
                   for k in range(KC)]

            def proj_blocks(w_t, nk, xs, m0, m1, evac):
                """For each feature block m in [m0,m1): accumulate over nk
                contraction tiles with the stationary weight serving all 4
                512-token chunks, then evacuate via `evac(m, ps_list)`."""
                for m in range(m0, m1):
                    ps = [psA.tile([P, 512], F32, name="ps", tag="ps")
                          for _ in range(NJ)]
                    for k in range(nk):
                        stat = w_t[:, k, m * P:(m + 1) * P]
                        for j in range(NJ):
                            nc.tensor.matmul(
                                ps[j][:], stat, xs[k][:, j * 512:(j + 1) * 512],
                                start=(k == 0), stop=(k == nk - 1))
                    evac(m, ps)

            def rope_evac(dst_tile):
                """Returns evac fn: raw rope block -> rotate+modulate -> dst."""
                def evac(m, ps):
                    raw = ropep.tile([P, L], F32R, name="raw", tag="raw")
                    for j in range(NJ):
                        nc.vector.tensor_copy(raw[:, j * 512:(j + 1) * 512],
                                              ps[j][:])
                    rawb = ropep.tile([P, L], BF16, name="rawb", tag="rawb")
                    for j in range(NJ):
                        nc.vector.tensor_copy(rawb[:, j * 512:(j + 1) * 512],
                                              ps[j][:])
                    for j in range(NJ):
                        sl = slice(j * 512, (j + 1) * 512)
                        rps = psA.tile([P, 512], F32, name="rps", tag="ps")
                        nc.tensor.matmul(rps[:], prot_t[:], rawb[:, sl],
                                         start=True, stop=True)
                        t1 = rtmp.tile([P, 512], F32, name="t1", tag="t1")
                        nc.vector.tensor_mul(t1[:], cos_t[:, sl], raw[:, sl])
                        t2 = rtmp.tile([P, 512], F32, name="t2", tag="t2")
                        nc.vector.tensor_mul(t2[:], sin_t[:, sl], rps[:])
                        nc.vector.tensor_add(dst_tile[:, sl], t1[:], t2[:])
                return evac

            # ================= Phase A: projections =========================
            with tc.tile_pool(name="xp", bufs=1) as xp:
                xs = [xp.tile([P, L], BF16, name="xt", tag=f"xt{k}")
                      for k in range(KD)]

                # --- A1: q_base + q_rope (folded weights; contraction = x)
                with tc.tile_pool(name="wqp", bufs=1) as wqp, \
                     tc.tile_pool(name="ropep", bufs=1) as ropep, \
                     tc.tile_pool(name="rtmp", bufs=2) as rtmp, \
                     tc.tile_pool(name="psA", bufs=8, space="PSUM") as psA:
                    # DMA issue order = dependency order of the first
                    # matmuls: q weights, then x, then the rope tables
                    # (first needed ~5 blocks in).
                    wq_t = wqp.tile([P, KD, DQB], BF16, name="wq_t", tag="wq")
                    nc.sync.dma_start(
                        out=wq_t[:], in_=wq.rearrange("(k p) j -> p k j", p=P))
                    for k in range(KD):
                        nc.sync.dma_start(out=xs[k][:],
                                          in_=xT[k * P:(k + 1) * P, :])
                    wqr_t = wqp.tile([P, KD, DQR], BF16, name="wqr_t", tag="wqr")
                    nc.sync.dma_start(
                        out=wqr_t[:], in_=wqr.rearrange("(k p) j -> p k j", p=P))
                    prot_t = xp.tile([P, P], BF16, name="prot_t", tag="prot")
                    nc.sync.dma_start(out=prot_t[:], in_=protT[:, :])
                    cos_t = xp.tile([P, L], F32, name="cos_t", tag="cos")
                    nc.sync.dma_start(out=cos_t[:], in_=cosr[:, :])
                    sin_t = xp.tile([P, L], F32, name="sin_t", tag="sin")
                    nc.sync.dma_start(out=sin_t[:], in_=sinr[:, :])

                    def evac_qb(m, ps):
                        for j in range(NJ):
                            nc.vector.tensor_copy(
                                qbT[m][:, j * 512:(j + 1) * 512], ps[j][:])
                    proj_blocks(wq_t, KD, xs, 0, NHG, evac_qb)
                    for m in range(2):
                        proj_blocks(wqr_t, KD, xs, m, m + 1,
                                    lambda _m, ps: rope_evac(qrT[m])(_m, ps))

                # --- A2: k_rope (+rope) and latent c
                with tc.tile_pool(name="wkp", bufs=1) as wkp, \
                     tc.tile_pool(name="ropep", bufs=1) as ropep, \
                     tc.tile_pool(name="rtmp", bufs=2) as rtmp, \
                     tc.tile_pool(name="psA", bufs=8, space="PSUM") as psA:
                    wkr_t = wkp.tile([P, KD, DQR], BF16, name="wkr_t", tag="wkr")
                    nc.sync.dma_start(
                        out=wkr_t[:], in_=wkr.rearrange("(k p) j -> p k j", p=P))
                    wkv_t = wkp.tile([P, KD, DC], BF16, name="wkv_t", tag="wkv")
                    nc.sync.dma_start(
                        out=wkv_t[:], in_=wkv.rearrange("(k p) j -> p k j", p=P))

                    for m in range(2):
                        proj_blocks(wkr_t, KD, xs, m, m + 1,
                                    lambda _m, ps: rope_evac(krT[m])(_m, ps))

                    def evac_c(m, ps):
                        for j in range(NJ):
                            nc.vector.tensor_copy(
                                cts[m][:, j * 512:(j + 1) * 512], ps[j][:])
                    proj_blocks(wkv_t, KD, xs, 0, KC, evac_c)

            # ---- pools for A3/B/C outputs (opened after x is freed)
            with tc.tile_pool(name="kvp", bufs=1) as kvp, \
                 tc.tile_pool(name="oTp", bufs=1) as oTp, \
                 tc.tile_pool(name="wop", bufs=1) as wop:
              kbT = [kvp.tile([P, L], BF16, name=f"kbT{h}", tag=f"kbT{h}")
                     for h in range(NHG)]
              vts = [kvp.tile([P, DQB], BF16, name=f"v{lt}", tag=f"v{lt}")
                     for lt in range(LK)]
              oT = [oTp.tile([P, L], BF16, name=f"oT{h}", tag=f"oT{h}")
                    for h in range(NHG)]

              # --- A3: k_base and v (contraction = c); x is freed
              with tc.tile_pool(name="wup", bufs=1) as wup, \
                   tc.tile_pool(name="psA", bufs=8, space="PSUM") as psA:
                wuk_t = wup.tile([P, KC, DQB], BF16, name="wuk_t", tag="wuk")
                nc.sync.dma_start(
                    out=wuk_t[:], in_=wuk.rearrange("(k p) j -> p k j", p=P))
                wuv_t = wup.tile([P, KC, DQB], BF16, name="wuv_t", tag="wuv")
                nc.sync.dma_start(
                    out=wuv_t[:], in_=wuv.rearrange("(k p) j -> p k j", p=P))

                def evac_kb(m, ps):
                    for j in range(NJ):
                        nc.vector.tensor_copy(
                            kbT[m][:, j * 512:(j + 1) * 512], ps[j][:])
                proj_blocks(wuk_t, KC, cts, 0, NHG, evac_kb)

                # v natural: stationary = c token-block, moving = W_U_V k-tile
                for lt in range(LK):
                    ps = psA.tile([P, DQB], F32, name="ps_v", tag="ps")
                    for k in range(KC):
                        nc.tensor.matmul(
                            ps[:], cts[k][:, lt * P:(lt + 1) * P], wuv_t[:, k, :],
                            start=(k == 0), stop=(k == KC - 1))
                    nc.vector.tensor_copy(vts[lt][:], ps[:])

                # prefetch W_O for phase C while B runs
                wo_t = wop.tile([P, NHG, D], BF16, name="wo_t", tag="wo")
                nc.sync.dma_start(
                    out=wo_t[:], in_=wo.rearrange("(k p) j -> p k j", p=P))

              # ================= Phase B: attention =========================
              with tc.tile_pool(name="ptp", bufs=1) as ptp, \
                   tc.tile_pool(name="rcp", bufs=2) as rcp, \
                   tc.tile_pool(name="psB", bufs=1, space="PSUM") as psB:
                ones_t = ptp.tile([P, P], BF16, name="ones_t", tag="ones")
                nc.sync.dma_start(out=ones_t[:], in_=onesd[:, :])
                for h in range(NHG):
                    qr_m, ro = qrT[h // 2], (h % 2) * DHR
                    kr_m = krT[h // 2]
                    ptb = [ptp.tile([P, LK, 512], BF16, name=f"ptb{lq}",
                                    tag=f"ptb{lq}") for lq in range(NJ)]
                    ot = [psB.tile([P, 512], F32, name="ot", tag="ot", bufs=4)
                          for _ in range(NJ)]
                    for lk in range(LK):
                        ksl = slice(lk * P, (lk + 1) * P)
                        for lq in range(NJ):
                            qsl = slice(lq * 512, (lq + 1) * 512)
                            st = psB.tile([P, 512], F32, name="st", tag="st",
                                          bufs=3)
                            nc.tensor.matmul(st[:], kbT[h][:, ksl],
                                             qbT[h][:, qsl],
                                             start=True, stop=False)
                            nc.tensor.matmul(
                                st[:], kr_m[ro:ro + DHR, ksl],
                                qr_m[ro:ro + DHR, qsl],
                                start=False, stop=True)
                            nc.scalar.activation(
                                ptb[lq][:, lk, :], st[:],
                                mybir.ActivationFunctionType.Exp, scale=SCALE)
                        for lq in range(NJ):
                            nc.tensor.matmul(
                                ot[lq][:], vts[lk][:, h * DH:(h + 1) * DH],
                                ptb[lq][:, lk, :],
                                start=(lk == 0), stop=(lk == LK - 1))
                    for lq in range(NJ):
                        # softmax denominator via ones-matmul (all output
                        # rows equal the key-sum), then normalize
                        rs = psB.tile([P, 512], F32, name="rs", tag="rs",
                                      bufs=1)
                        for lk in range(LK):
                            nc.tensor.matmul(
                                rs[:], ones_t[:], ptb[lq][:, lk, :],
                                start=(lk == 0), stop=(lk == LK - 1))
                        rec = rcp.tile([P, 512], F32, name="rec", tag="rec")
                        nc.vector.reciprocal(rec[:], rs[:])
                        nc.vector.tensor_mul(
                            oT[h][:, lq * 512:(lq + 1) * 512], ot[lq][:],
                            rec[:])

              # ================= Phase C: output projection =================
              with tc.tile_pool(name="ocp", bufs=4) as ocp, \
                   tc.tile_pool(name="psC", bufs=8, space="PSUM") as psC:
                for mt in range(L // P):
                    msl = slice(mt * P, (mt + 1) * P)
                    ps = [psC.tile([P, 512], F32, name="ps_o", tag="psc")
                          for _ in range(NJ)]
                    for k in range(NHG):
                        stat = oT[k][:, msl]
                        for nt in range(NJ):
                            nc.tensor.matmul(
                                ps[nt][:], stat, wo_t[:, k, nt * 512:(nt + 1) * 512],
                                start=(k == 0), stop=(k == NHG - 1))
                    oc = ocp.tile([P, D], BF16, name="oc", tag="oc")
                    for nt in range(NJ):
                        nc.vector.tensor_copy(oc[:, nt * 512:(nt + 1) * 512],
                                              ps[nt][:])
                    nc.sync.dma_start(out=out[msl, :], in_=oc[:])

    nc.compile()
    return nc


def _rope_tables():
    """cos/sin in transposed, 2-head-replicated layout (128 x L), plus Prot^T."""
    inv_freq = 1.0 / (ROPE_THETA ** (np.arange(0, DHR, 2, dtype=np.float32) / DHR))
    ang = np.arange(L, dtype=np.float32)[:, None] * inv_freq[None, :]  # (L, 32)
    cos64 = np.concatenate([np.cos(ang), np.cos(ang)], axis=1).T  # (64, L)
    sin64 = np.concatenate([np.sin(ang), np.sin(ang)], axis=1).T
    cosr = np.ascontiguousarray(np.tile(cos64, (2, 1)), dtype=np.float32)
    sinr = np.ascontiguousarray(np.tile(sin64, (2, 1)), dtype=np.float32)
    # rot(x) = [-x2, x1] per 64-dim head; block-diag over 2 heads; transposed.
    p64 = np.zeros((DHR, DHR), dtype=np.float32)
    half = DHR // 2
    p64[np.arange(half), np.arange(half) + half] = -1.0
    p64[np.arange(half) + half, np.arange(half)] = 1.0
    p128 = np.zeros((P, P), dtype=np.float32)
    p128[:DHR, :DHR] = p64
    p128[DHR:, DHR:] = p64
    protT = np.ascontiguousarray(p128.T)
    return cosr, sinr, protT


def _bf16():
    return mybir.dt.np(BF16)


def build_in_maps(x, W_D_Q, W_U_Q, W_Q_R, W_D_KV, W_U_K, W_K_R, W_U_V, W_O):
    cosr, sinr, protT = _rope_tables()
    bf = _bf16()
    f = np.float32
    x = np.asarray(x, f)
    W_D_Q = np.asarray(W_D_Q, f)
    # fold the low-rank query compression into single input->head projections
    Wq_full = (W_D_Q @ np.asarray(W_U_Q, f)).astype(bf)     # (D, NH*DH)
    Wqr_full = (W_D_Q @ np.asarray(W_Q_R, f)).astype(bf)    # (D, NH*DHR)
    xTs = [np.ascontiguousarray(x[b].T).astype(bf) for b in range(B)]
    wkv_b = np.ascontiguousarray(np.asarray(W_D_KV, f)).astype(bf)
    in_maps = []
    for c in range(8):
        b, g = c // 4, c % 4
        hb = slice(g * DQB, (g + 1) * DQB)
        hr = slice(g * DQR, (g + 1) * DQR)
        in_maps.append(dict(
            xT=xTs[b],
            wq=np.ascontiguousarray(Wq_full[:, hb]),
            wqr=np.ascontiguousarray(Wqr_full[:, hr]),
            wkv=wkv_b,
            wuk=np.ascontiguousarray(np.asarray(W_U_K, f)[:, hb]).astype(bf),
            wkr=np.ascontiguousarray(np.asarray(W_K_R, f)[:, hr]).astype(bf),
            wuv=np.ascontiguousarray(np.asarray(W_U_V, f)[:, hb]).astype(bf),
            wo=np.ascontiguousarray(np.asarray(W_O, f)[hb, :]).astype(bf),
            cosr=cosr, sinr=sinr,
            protT=protT.astype(bf),
            onesd=np.ones((P, P), dtype=bf),
        ))
    return in_maps


def kernel(x, W_D_Q, W_U_Q, W_Q_R, W_D_KV, W_U_K, W_K_R, W_U_V, W_O):
    if "nc" not in _CACHED:
        _CACHED["nc"] = _build()
    nc = _CACHED["nc"]
    in_maps = build_in_maps(x, W_D_Q, W_U_Q, W_Q_R, W_D_KV, W_U_K, W_K_R,
                            W_U_V, W_O)
    res = run_bass_kernel_spmd(nc, in_maps, core_ids=list(range(8)))
    outs = [r["out"].astype(np.float32) for r in res.results]
    full = np.stack(
        [outs[b * 4] + outs[b * 4 + 1] + outs[b * 4 + 2] + outs[b * 4 + 3]
         for b in range(B)]).astype(np.float32)
    return full


# revision 17
# speedup vs baseline: 30.3332x; 1.1799x over previous
"""Multi-Head Latent Attention (MLA) Trainium2 kernel, 8-way sharded. v2.

Sharding: 8 cores = 2 (batch) x 4 (head groups of 4 heads). Host sums the
4 partial output projections per batch element.

v2 vs v1:
  - bf16 everywhere on the PE (FWL halves LDWEIGHTS; DMA bytes halved).
  - Q path folded on host: Wq = W_D_Q @ W_U_Q[:,hb]  (saves the qc matmul,
    -5.6e9 MACs/core).
  - No DRAM spills: x, q/k/v, c all SBUF-resident for the whole kernel.
  - Loop nests keep each PE stationary operand for 4x512 moving rows.
  - Softmax row-sums on GpSimd (partition reduce) instead of a ones-matmul.
  - PSUM evacuations pinned to the Vector engine (ACT does only exp).

Everything is computed TRANSPOSED (feature dim on partitions): scores come
out as S^T (keys on partitions), so softmax = plain exp, normalization via
GpSimd partition-sum + reciprocal + broadcast.
"""

import sys

sys.path.insert(0, "/opt/trn_rl_repo")

import numpy as np

import concourse.bacc as bacc
import concourse.mybir as mybir
import concourse.tile as tile
from concourse import bass_isa
from concourse.bass_utils import run_bass_kernel_spmd

# Problem dims (hardcoded per contract)
D, NH, DH, DC, DCQ, DHR = 2048, 16, 128, 512, 1536, 64
B, L = 2, 2048
ROPE_THETA = 10000.0

NHG = 4                 # heads per core
DQB = NHG * DH          # 512: per-core base q/k feature dim (also v dim)
DQR = NHG * DHR         # 256: per-core rope feature dim
P = 128
SCALE = DH ** -0.5

F32 = mybir.dt.float32
F32R = mybir.dt.float32r
BF16 = mybir.dt.bfloat16

KD = D // P            # 16 x k-tiles
KC = DC // P           # 4  c k-tiles
NJ = L // 512          # 4  512-token chunks
LK = L // P            # 16 key tiles

_CACHED = {}


def _build():
    nc = bacc.Bacc("TRN2", target_bir_lowering=False, debug=False)

    # ---- DRAM I/O (per-core data; program is SPMD)
    xT = nc.dram_tensor("xT", [D, L], BF16, kind="ExternalInput")
    wq = nc.dram_tensor("wq", [D, DQB], BF16, kind="ExternalInput")
    wqr = nc.dram_tensor("wqr", [D, DQR], BF16, kind="ExternalInput")
    wkv = nc.dram_tensor("wkv", [D, DC], BF16, kind="ExternalInput")
    wuk = nc.dram_tensor("wuk", [DC, DQB], BF16, kind="ExternalInput")
    wkr = nc.dram_tensor("wkr", [D, DQR], BF16, kind="ExternalInput")
    wuv = nc.dram_tensor("wuv", [DC, DQB], BF16, kind="ExternalInput")
    wo = nc.dram_tensor("wo", [DQB, D], BF16, kind="ExternalInput")
    cosr = nc.dram_tensor("cosr", [P, L], F32, kind="ExternalInput")
    sinr = nc.dram_tensor("sinr", [P, L], F32, kind="ExternalInput")
    protT = nc.dram_tensor("protT", [P, P], BF16, kind="ExternalInput")
    onesd = nc.dram_tensor("onesd", [P, P], BF16, kind="ExternalInput")
    out = nc.dram_tensor("out", [L, D], BF16, kind="ExternalOutput")

    with tile.TileContext(nc) as tc:
        with tc.tile_pool(name="persist", bufs=1) as pp, \
             tc.tile_pool(name="cpool", bufs=1) as cpool:

            # projections computed in phase A1/A2, live until phase B
            qbT = [pp.tile([P, L], BF16, name=f"qbT{h}", tag=f"qbT{h}")
                   for h in range(NHG)]
            qrT = [pp.tile([P, L], BF16, name=f"qrT{m}", tag=f"qrT{m}")
                   for m in range(2)]
            krT = [pp.tile([P, L], BF16, name=f"krT{m}", tag=f"krT{m}")
                   for m in range(2)]
            cts = [cpool.tile([P, L], BF16, name=f"c{k}", tag=f"c{k}")
                   for k in range(KC)]

            def proj_blocks(w_t, nk, xs, m0, m1, evac):
                """For each feature block m in [m0,m1): accumulate over nk
                contraction tiles with the stationary weight serving all 4
                512-token chunks, then evacuate via `evac(m, ps_list)`."""
                for m in range(m0, m1):
                    ps = [psA.tile([P, 512], F32, name="ps", tag="ps")
                          for _ in range(NJ)]
                    for k in range(nk):
                        stat = w_t[:, k, m * P:(m + 1) * P]
                        for j in range(NJ):
                            nc.tensor.matmul(
                                ps[j][:], stat, xs[k][:, j * 512:(j + 1) * 512],
                                start=(k == 0), stop=(k == nk - 1))
                    evac(m, ps)

            def rope_evac(dst_tile):
                """Returns evac fn: raw rope block -> rotate+modulate -> dst."""
                def evac(m, ps):
                    raw = ropep.tile([P, L], F32R, name="raw", tag="raw")
                    for j in range(NJ):
                        nc.vector.tensor_copy(raw[:, j * 512:(j + 1) * 512],
                                              ps[j][:])
                    rawb = ropep.tile([P, L], BF16, name="rawb", tag="rawb")
                    for j in range(NJ):
                        nc.vector.tensor_copy(rawb[:, j * 512:(j + 1) * 512],
                                              ps[j][:])
                    for j in range(NJ):
                        sl = slice(j * 512, (j + 1) * 512)
                        rps = psA.tile([P, 512], F32, name="rps", tag="ps")
                        nc.tensor.matmul(rps[:], prot_t[:], rawb[:, sl],
                                         start=True, stop=True)
                        t1 = rtmp.tile([P, 512], F32, name="t1", tag="t1")
                        nc.vector.tensor_mul(t1[:], cos_t[:, sl], raw[:, sl])
                        t2 = rtmp.tile([P, 512], F32, name="t2", tag="t2")
                        nc.vector.tensor_mul(t2[:], sin_t[:, sl], rps[:])
                        nc.vector.tensor_add(dst_tile[:, sl], t1[:], t2[:])
                return evac

            # ================= Phase A: projections =========================
            with tc.tile_pool(name="xp", bufs=1) as xp:
                xs = [xp.tile([P, L], BF16, name="xt", tag=f"xt{k}")
                      for k in range(KD)]

                # --- A1: q_base + q_rope (folded weights; contraction = x)
                with tc.tile_pool(name="wqp", bufs=1) as wqp, \
                     tc.tile_pool(name="ropep", bufs=1) as ropep, \
                     tc.tile_pool(name="rtmp", bufs=2) as rtmp, \
                     tc.tile_pool(name="psA", bufs=8, space="PSUM") as psA:
                    # DMA issue order = dependency order of the first
                    # matmuls: q weights, then x, then the rope tables
                    # (first needed ~5 blocks in).
                    wq_t = wqp.tile([P, KD, DQB], BF16, name="wq_t", tag="wq")
                    nc.sync.dma_start(
                        out=wq_t[:], in_=wq.rearrange("(k p) j -> p k j", p=P))
                    for k in range(KD):
                        nc.sync.dma_start(out=xs[k][:],
                                          in_=xT[k * P:(k + 1) * P, :])
                    wqr_t = wqp.tile([P, KD, DQR], BF16, name="wqr_t", tag="wqr")
                    nc.sync.dma_start(
                        out=wqr_t[:], in_=wqr.rearrange("(k p) j -> p k j", p=P))
                    prot_t = xp.tile([P, P], BF16, name="prot_t", tag="prot")
                    nc.sync.dma_start(out=prot_t[:], in_=protT[:, :])
                    cos_t = xp.tile([P, L], F32, name="cos_t", tag="cos")
                    nc.sync.dma_start(out=cos_t[:], in_=cosr[:, :])
                    sin_t = xp.tile([P, L], F32, name="sin_t", tag="sin")
                    nc.sync.dma_start(out=sin_t[:], in_=sinr[:, :])

                    def evac_qb(m, ps):
                        for j in range(NJ):
                            nc.vector.tensor_copy(
                                qbT[m][:, j * 512:(j + 1) * 512], ps[j][:])
                    proj_blocks(wq_t, KD, xs, 0, NHG, evac_qb)
                    for m in range(2):
                        proj_blocks(wqr_t, KD, xs, m, m + 1,
                                    lambda _m, ps: rope_evac(qrT[m])(_m, ps))

                # --- A2: k_rope (+rope) and latent c
                with tc.tile_pool(name="wkp", bufs=1) as wkp, \
                     tc.tile_pool(name="ropep", bufs=1) as ropep, \
                     tc.tile_pool(name="rtmp", bufs=2) as rtmp, \
                     tc.tile_pool(name="psA", bufs=8, space="PSUM") as psA:
                    wkr_t = wkp.tile([P, KD, DQR], BF16, name="wkr_t", tag="wkr")
                    nc.sync.dma_start(
                        out=wkr_t[:], in_=wkr.rearrange("(k p) j -> p k j", p=P))
                    wkv_t = wkp.tile([P, KD, DC], BF16, name="wkv_t", tag="wkv")
                    nc.sync.dma_start(
                        out=wkv_t[:], in_=wkv.rearrange("(k p) j -> p k j", p=P))

                    for m in range(2):
                        proj_blocks(wkr_t, KD, xs, m, m + 1,
                                    lambda _m, ps: rope_evac(krT[m])(_m, ps))

                    def evac_c(m, ps):
                        for j in range(NJ):
                            nc.vector.tensor_copy(
                                cts[m][:, j * 512:(j + 1) * 512], ps[j][:])
                    proj_blocks(wkv_t, KD, xs, 0, KC, evac_c)

            # ---- pools for A3/B/C outputs (opened after x is freed)
            with tc.tile_pool(name="kvp", bufs=1) as kvp, \
                 tc.tile_pool(name="oTp", bufs=1) as oTp, \
                 tc.tile_pool(name="wop", bufs=1) as wop:
              kbT = [kvp.tile([P, L], BF16, name=f"kbT{h}", tag=f"kbT{h}")
                     for h in range(NHG)]
              vts = [kvp.tile([P, DQB], BF16, name=f"v{lt}", tag=f"v{lt}")
                     for lt in range(LK)]
              oT = [oTp.tile([P, L], BF16, name=f"oT{h}", tag=f"oT{h}")
                    for h in range(NHG)]

              # --- A3: k_base and v (contraction = c); x is freed
              with tc.tile_pool(name="wup", bufs=1) as wup, \
                   tc.tile_pool(name="psA", bufs=8, space="PSUM") as psA:
                wuk_t = wup.tile([P, KC, DQB], BF16, name="wuk_t", tag="wuk")
                nc.sync.dma_start(
                    out=wuk_t[:], in_=wuk.rearrange("(k p) j -> p k j", p=P))
                wuv_t = wup.tile([P, KC, DQB], BF16, name="wuv_t", tag="wuv")
                nc.sync.dma_start(
                    out=wuv_t[:], in_=wuv.rearrange("(k p) j -> p k j", p=P))

                def evac_kb(m, ps):
                    for j in range(NJ):
                        nc.vector.tensor_copy(
                            kbT[m][:, j * 512:(j + 1) * 512], ps[j][:])
                proj_blocks(wuk_t, KC, cts, 0, NHG, evac_kb)

                # v natural: stationary = c token-block, moving = W_U_V k-tile
                for lt in range(LK):
                    ps = psA.tile([P, DQB], F32, name="ps_v", tag="ps")
                    for k in range(KC):
                        nc.tensor.matmul(
                            ps[:], cts[k][:, lt * P:(lt + 1) * P], wuv_t[:, k, :],
                            start=(k == 0), stop=(k == KC - 1))
                    nc.vector.tensor_copy(vts[lt][:], ps[:])

                # prefetch W_O for phase C while B runs
                wo_t = wop.tile([P, NHG, D], BF16, name="wo_t", tag="wo")
                nc.sync.dma_start(
                    out=wo_t[:], in_=wo.rearrange("(k p) j -> p k j", p=P))

              # ================= Phase B: attention =========================
              with tc.tile_pool(name="ptp", bufs=1) as ptp, \
                   tc.tile_pool(name="rcp", bufs=2) as rcp, \
                   tc.tile_pool(name="psB", bufs=1, space="PSUM") as psB:
                ones_t = ptp.tile([P, P], BF16, name="ones_t", tag="ones")
                nc.sync.dma_start(out=ones_t[:], in_=onesd[:, :])
                LAG = 2  # PV trails exp by LAG key-tiles so PE never waits ACT
                for h in range(NHG):
                    qr_m, ro = qrT[h // 2], (h % 2) * DHR
                    kr_m = krT[h // 2]
                    # pt buffered per lq-PAIR: [keys, lk, 1024] so exp runs
                    # on 1024-wide tiles (half the ACT per-op overhead)
                    ptb = [ptp.tile([P, LK, 1024], BF16, name=f"ptb{pr}",
                                    tag=f"ptb{pr}") for pr in range(2)]
                    ot = [psB.tile([P, 512], F32, name="ot", tag="ot", bufs=4)
                          for _ in range(NJ)]

                    def pv(j):
                        for lq in range(NJ):
                            nc.tensor.matmul(
                                ot[lq][:], vts[j][:, h * DH:(h + 1) * DH],
                                ptb[lq // 2][:, j,
                                             (lq % 2) * 512:(lq % 2) * 512 + 512],
                                start=(j == 0), stop=(j == LK - 1))

                    for lk in range(LK):
                        ksl = slice(lk * P, (lk + 1) * P)
                        sts = [psB.tile([P, 1024], F32, name="st", tag="st",
                                        bufs=2) for _ in range(2)]
                        for lq in range(NJ):   # kb stationary held for 4 mms
                            nc.tensor.matmul(
                                sts[lq // 2][:, (lq % 2) * 512:(lq % 2) * 512 + 512],
                                kbT[h][:, ksl],
                                qbT[h][:, lq * 512:(lq + 1) * 512],
                                start=True, stop=False)
                        for lq in range(NJ):   # kr stationary held for 4 mms
                            nc.tensor.matmul(
                                sts[lq // 2][:, (lq % 2) * 512:(lq % 2) * 512 + 512],
                                kr_m[ro:ro + DHR, ksl],
                                qr_m[ro:ro + DHR, lq * 512:(lq + 1) * 512],
                                start=False, stop=True)
                        for pr in range(2):
                            nc.scalar.activation(
                                ptb[pr][:, lk, :], sts[pr][:],
                                mybir.ActivationFunctionType.Exp, scale=SCALE)
                        if lk >= LAG:
                            pv(lk - LAG)
                    for j in range(LK - LAG, LK):
                        pv(j)
                    for lq in range(NJ):
                        # softmax denominator via ones-matmul (all output
                        # rows equal the key-sum), then normalize
                        rs = psB.tile([P, 1024], F32, name="rs", tag="st",
                                      bufs=2)
                        for lk in range(LK):
                            nc.tensor.matmul(
                                rs[:, :512], ones_t[:],
                                ptb[lq // 2][:, lk,
                                             (lq % 2) * 512:(lq % 2) * 512 + 512],
                                start=(lk == 0), stop=(lk == LK - 1))
                        rec = rcp.tile([P, 512], F32, name="rec", tag="rec")
                        nc.vector.reciprocal_approx_fast(rec[:], rs[:, :512])
                        nc.vector.tensor_mul(
                            oT[h][:, lq * 512:(lq + 1) * 512], ot[lq][:],
                            rec[:])

              # ================= Phase C: output projection =================
              with tc.tile_pool(name="ocp", bufs=4) as ocp, \
                   tc.tile_pool(name="psC", bufs=8, space="PSUM") as psC:
                for mt in range(L // P):
                    msl = slice(mt * P, (mt + 1) * P)
                    ps = [psC.tile([P, 512], F32, name="ps_o", tag="psc")
                          for _ in range(NJ)]
                    for k in range(NHG):
                        stat = oT[k][:, msl]
                        for nt in range(NJ):
                            nc.tensor.matmul(
                                ps[nt][:], stat, wo_t[:, k, nt * 512:(nt + 1) * 512],
                                start=(k == 0), stop=(k == NHG - 1))
                    oc = ocp.tile([P, D], BF16, name="oc", tag="oc")
                    for nt in range(NJ):
                        nc.vector.tensor_copy(oc[:, nt * 512:(nt + 1) * 512],
                                              ps[nt][:])
                    nc.sync.dma_start(out=out[msl, :], in_=oc[:])

    nc.compile()
    return nc


def _rope_tables():
    """cos/sin in transposed, 2-head-replicated layout (128 x L), plus Prot^T."""
    inv_freq = 1.0 / (ROPE_THETA ** (np.arange(0, DHR, 2, dtype=np.float32) / DHR))
    ang = np.arange(L, dtype=np.float32)[:, None] * inv_freq[None, :]  # (L, 32)
    cos64 = np.concatenate([np.cos(ang), np.cos(ang)], axis=1).T  # (64, L)
    sin64 = np.concatenate([np.sin(ang), np.sin(ang)], axis=1).T
    cosr = np.ascontiguousarray(np.tile(cos64, (2, 1)), dtype=np.float32)
    sinr = np.ascontiguousarray(np.tile(sin64, (2, 1)), dtype=np.float32)
    # rot(x) = [-x2, x1] per 64-dim head; block-diag over 2 heads; transposed.
    p64 = np.zeros((DHR, DHR), dtype=np.float32)
    half = DHR // 2
    p64[np.arange(half), np.arange(half) + half] = -1.0
    p64[np.arange(half) + half, np.arange(half)] = 1.0
    p128 = np.zeros((P, P), dtype=np.float32)
    p128[:DHR, :DHR] = p64
    p128[DHR:, DHR:] = p64
    protT = np.ascontiguousarray(p128.T)
    return cosr, sinr, protT


def _bf16():
    return mybir.dt.np(BF16)


def build_in_maps(x, W_D_Q, W_U_Q, W_Q_R, W_D_KV, W_U_K, W_K_R, W_U_V, W_O):
    cosr, sinr, protT = _rope_tables()
    bf = _bf16()
    f = np.float32
    x = np.asarray(x, f)
    W_D_Q = np.asarray(W_D_Q, f)
    # fold the low-rank query compression into single input->head projections
    Wq_full = (W_D_Q @ np.asarray(W_U_Q, f)).astype(bf)     # (D, NH*DH)
    Wqr_full = (W_D_Q @ np.asarray(W_Q_R, f)).astype(bf)    # (D, NH*DHR)
    xTs = [np.ascontiguousarray(x[b].T).astype(bf) for b in range(B)]
    wkv_b = np.ascontiguousarray(np.asarray(W_D_KV, f)).astype(bf)
    in_maps = []
    for c in range(8):
        b, g = c // 4, c % 4
        hb = slice(g * DQB, (g + 1) * DQB)
        hr = slice(g * DQR, (g + 1) * DQR)
        in_maps.append(dict(
            xT=xTs[b],
            wq=np.ascontiguousarray(Wq_full[:, hb]),
            wqr=np.ascontiguousarray(Wqr_full[:, hr]),
            wkv=wkv_b,
            wuk=np.ascontiguousarray(np.asarray(W_U_K, f)[:, hb]).astype(bf),
            wkr=np.ascontiguousarray(np.asarray(W_K_R, f)[:, hr]).astype(bf),
            wuv=np.ascontiguousarray(np.asarray(W_U_V, f)[:, hb]).astype(bf),
            wo=np.ascontiguousarray(np.asarray(W_O, f)[hb, :]).astype(bf),
            cosr=cosr, sinr=sinr,
            protT=protT.astype(bf),
            onesd=np.ones((P, P), dtype=bf),
        ))
    return in_maps


def kernel(x, W_D_Q, W_U_Q, W_Q_R, W_D_KV, W_U_K, W_K_R, W_U_V, W_O):
    if "nc" not in _CACHED:
        _CACHED["nc"] = _build()
    nc = _CACHED["nc"]
    in_maps = build_in_maps(x, W_D_Q, W_U_Q, W_Q_R, W_D_KV, W_U_K, W_K_R,
                            W_U_V, W_O)
    res = run_bass_kernel_spmd(nc, in_maps, core_ids=list(range(8)))
    outs = [r["out"].astype(np.float32) for r in res.results]
    full = np.stack(
        [outs[b * 4] + outs[b * 4 + 1] + outs[b * 4 + 2] + outs[b * 4 + 3]
         for b in range(B)]).astype(np.float32)
    return full


# revision 24
# speedup vs baseline: 32.9288x; 1.0856x over previous
"""Multi-Head Latent Attention (MLA) Trainium2 kernel, 8-way sharded. v2.

Sharding: 8 cores = 2 (batch) x 4 (head groups of 4 heads). Host sums the
4 partial output projections per batch element.

v2 vs v1:
  - bf16 everywhere on the PE (FWL halves LDWEIGHTS; DMA bytes halved).
  - Q path folded on host: Wq = W_D_Q @ W_U_Q[:,hb]  (saves the qc matmul,
    -5.6e9 MACs/core).
  - No DRAM spills: x, q/k/v, c all SBUF-resident for the whole kernel.
  - Loop nests keep each PE stationary operand for 4x512 moving rows.
  - Softmax row-sums on GpSimd (partition reduce) instead of a ones-matmul.
  - PSUM evacuations pinned to the Vector engine (ACT does only exp).

Everything is computed TRANSPOSED (feature dim on partitions): scores come
out as S^T (keys on partitions), so softmax = plain exp, normalization via
GpSimd partition-sum + reciprocal + broadcast.
"""

import sys

sys.path.insert(0, "/opt/trn_rl_repo")

import numpy as np

import concourse.bacc as bacc
import concourse.mybir as mybir
import concourse.tile as tile
from concourse import bass_isa
from concourse.bass_utils import run_bass_kernel_spmd

# Problem dims (hardcoded per contract)
D, NH, DH, DC, DCQ, DHR = 2048, 16, 128, 512, 1536, 64
B, L = 2, 2048
ROPE_THETA = 10000.0

NHG = 4                 # heads per core
DQB = NHG * DH          # 512: per-core base q/k feature dim (also v dim)
DQR = NHG * DHR         # 256: per-core rope feature dim
P = 128
SCALE = DH ** -0.5

F32 = mybir.dt.float32
F32R = mybir.dt.float32r
BF16 = mybir.dt.bfloat16

KD = D // P            # 16 x k-tiles
KC = DC // P           # 4  c k-tiles
NJ = L // 512          # 4  512-token chunks
LK = L // P            # 16 key tiles

_CACHED = {}


def _build():
    nc = bacc.Bacc("TRN2", target_bir_lowering=False, debug=False)

    # ---- DRAM I/O (per-core data; program is SPMD)
    xT = nc.dram_tensor("xT", [D, L], BF16, kind="ExternalInput")
    wq = nc.dram_tensor("wq", [D, DQB], BF16, kind="ExternalInput")
    wqr = nc.dram_tensor("wqr", [D, DQR], BF16, kind="ExternalInput")
    wkv = nc.dram_tensor("wkv", [D, DC], BF16, kind="ExternalInput")
    wuk = nc.dram_tensor("wuk", [DC, DQB], BF16, kind="ExternalInput")
    wkr = nc.dram_tensor("wkr", [D, DQR], BF16, kind="ExternalInput")
    wuv = nc.dram_tensor("wuv", [DC, DQB], BF16, kind="ExternalInput")
    wo = nc.dram_tensor("wo", [DQB, D], BF16, kind="ExternalInput")
    cosr = nc.dram_tensor("cosr", [P, L], F32, kind="ExternalInput")
    sinr = nc.dram_tensor("sinr", [P, L], F32, kind="ExternalInput")
    protT = nc.dram_tensor("protT", [P, P], BF16, kind="ExternalInput")
    onesf = nc.dram_tensor("onesf", [P, P], F32R, kind="ExternalInput")
    out = nc.dram_tensor("out", [L, D], BF16, kind="ExternalOutput")

    with tile.TileContext(nc) as tc:
        with tc.tile_pool(name="persist", bufs=1) as pp, \
             tc.tile_pool(name="cpool", bufs=1) as cpool:

            # projections computed in phase A1/A2, live until phase B
            qbT = [pp.tile([P, L], BF16, name=f"qbT{h}", tag=f"qbT{h}")
                   for h in range(NHG)]
            qrT = [pp.tile([P, L], BF16, name=f"qrT{m}", tag=f"qrT{m}")
                   for m in range(2)]
            krT = [pp.tile([P, L], BF16, name=f"krT{m}", tag=f"krT{m}")
                   for m in range(2)]
            cts = [cpool.tile([P, L], BF16, name=f"c{k}", tag=f"c{k}")
                   for k in range(KC)]

            def proj_blocks(w_t, nk, xs, m0, m1, evac):
                """For each feature block m in [m0,m1): accumulate over nk
                contraction tiles with the stationary weight serving all 4
                512-token chunks, then evacuate via `evac(m, ps_list)`."""
                for m in range(m0, m1):
                    ps = [psA.tile([P, 512], F32, name="ps", tag="ps")
                          for _ in range(NJ)]
                    for k in range(nk):
                        stat = w_t[:, k, m * P:(m + 1) * P]
                        for j in range(NJ):
                            nc.tensor.matmul(
                                ps[j][:], stat, xs[k][:, j * 512:(j + 1) * 512],
                                start=(k == 0), stop=(k == nk - 1))
                    evac(m, ps)

            def rope_evac(dst_tile):
                """Returns evac fn: raw rope block -> rotate+modulate -> dst.

                Emission order is chosen so the proj PSUM tiles ps[j] free up
                early (cast + t1 lead on DVE per j), and the t2/fin tail
                drains on DVE while the PE runs the next proj block."""
                def evac(m, ps):
                    rawb = ropep.tile([P, L], BF16, name="rawb", tag="rawb")
                    stash = []
                    for j in range(NJ):
                        sl = slice(j * 512, (j + 1) * 512)
                        nc.vector.tensor_copy(rawb[:, sl], ps[j][:])
                        rps = psA.tile([P, 512], F32, name="rps", tag="ps")
                        nc.tensor.matmul(rps[:], prot_t[:], rawb[:, sl],
                                         start=True, stop=True)
                        t1 = rtmp.tile([P, 512], F32, name="t1", tag="t1",
                                       bufs=4)
                        nc.vector.tensor_mul(t1[:], cos_t[:, sl], ps[j][:])
                        stash.append((sl, rps, t1))
                    for sl, rps, t1 in stash:
                        t2 = rtmp.tile([P, 512], F32, name="t2", tag="t2")
                        nc.vector.tensor_mul(t2[:], sin_t[:, sl], rps[:])
                        nc.vector.tensor_add(dst_tile[:, sl], t1[:], t2[:])
                return evac

            # ================= Phase A: projections =========================
            with tc.tile_pool(name="xp", bufs=1) as xp:
                xs = [xp.tile([P, L], BF16, name="xt", tag=f"xt{k}")
                      for k in range(KD)]

                # --- A1: q_base + q_rope (folded weights; contraction = x)
                with tc.tile_pool(name="wqp", bufs=1) as wqp, \
                     tc.tile_pool(name="ropep", bufs=1) as ropep, \
                     tc.tile_pool(name="rtmp", bufs=2) as rtmp, \
                     tc.tile_pool(name="psA", bufs=8, space="PSUM") as psA:
                    # DMA issue order = dependency order of the first
                    # matmuls: q weights (per k-tile, so matmul k waits only
                    # its own slice), then x, then the rope tables (first
                    # needed ~5 blocks in).
                    wq_t = wqp.tile([P, KD, DQB], BF16, name="wq_t", tag="wq")
                    for k in range(KD):
                        nc.sync.dma_start(out=wq_t[:, k, :],
                                          in_=wq[k * P:(k + 1) * P, :])
                        nc.sync.dma_start(out=xs[k][:],
                                          in_=xT[k * P:(k + 1) * P, :])
                    wqr_t = wqp.tile([P, KD, DQR], BF16, name="wqr_t", tag="wqr")
                    nc.sync.dma_start(
                        out=wqr_t[:], in_=wqr.rearrange("(k p) j -> p k j", p=P))
                    prot_t = xp.tile([P, P], BF16, name="prot_t", tag="prot")
                    nc.sync.dma_start(out=prot_t[:], in_=protT[:, :])
                    cos_t = xp.tile([P, L], F32, name="cos_t", tag="cos")
                    nc.sync.dma_start(out=cos_t[:], in_=cosr[:, :])
                    sin_t = xp.tile([P, L], F32, name="sin_t", tag="sin")
                    nc.sync.dma_start(out=sin_t[:], in_=sinr[:, :])

                    def evac_qb(m, ps):
                        for j in range(NJ):
                            nc.vector.tensor_copy(
                                qbT[m][:, j * 512:(j + 1) * 512], ps[j][:])
                    proj_blocks(wq_t, KD, xs, 0, NHG, evac_qb)
                    for m in range(2):
                        proj_blocks(wqr_t, KD, xs, m, m + 1,
                                    lambda _m, ps: rope_evac(qrT[m])(_m, ps))

                # --- A2: k_rope (+rope) and latent c
                with tc.tile_pool(name="wkp", bufs=1) as wkp, \
                     tc.tile_pool(name="ropep", bufs=1) as ropep, \
                     tc.tile_pool(name="rtmp", bufs=2) as rtmp, \
                     tc.tile_pool(name="psA", bufs=8, space="PSUM") as psA:
                    wkr_t = wkp.tile([P, KD, DQR], BF16, name="wkr_t", tag="wkr")
                    nc.sync.dma_start(
                        out=wkr_t[:], in_=wkr.rearrange("(k p) j -> p k j", p=P))
                    wkv_t = wkp.tile([P, KD, DC], BF16, name="wkv_t", tag="wkv")
                    nc.sync.dma_start(
                        out=wkv_t[:], in_=wkv.rearrange("(k p) j -> p k j", p=P))

                    for m in range(2):
                        proj_blocks(wkr_t, KD, xs, m, m + 1,
                                    lambda _m, ps: rope_evac(krT[m])(_m, ps))

                    def evac_c(m, ps):
                        for j in range(NJ):
                            nc.vector.tensor_copy(
                                cts[m][:, j * 512:(j + 1) * 512], ps[j][:])
                    proj_blocks(wkv_t, KD, xs, 0, KC, evac_c)

            # ---- pools for A3/B/C outputs (opened after x is freed)
            with tc.tile_pool(name="kvp", bufs=1) as kvp, \
                 tc.tile_pool(name="oTp", bufs=1) as oTp, \
                 tc.tile_pool(name="wop", bufs=1) as wop:
              kbT = [kvp.tile([P, L], BF16, name=f"kbT{h}", tag=f"kbT{h}")
                     for h in range(NHG)]
              vts = [kvp.tile([P, DQB], BF16, name=f"v{lt}", tag=f"v{lt}")
                     for lt in range(LK)]
              oT = [oTp.tile([P, L], BF16, name=f"oT{h}", tag=f"oT{h}")
                    for h in range(NHG)]

              # --- A3: k_base and v (contraction = c); x is freed
              with tc.tile_pool(name="wup", bufs=1) as wup, \
                   tc.tile_pool(name="psA", bufs=8, space="PSUM") as psA:
                wuk_t = wup.tile([P, KC, DQB], BF16, name="wuk_t", tag="wuk")
                nc.sync.dma_start(
                    out=wuk_t[:], in_=wuk.rearrange("(k p) j -> p k j", p=P))
                wuv_t = wup.tile([P, KC, DQB], BF16, name="wuv_t", tag="wuv")
                nc.sync.dma_start(
                    out=wuv_t[:], in_=wuv.rearrange("(k p) j -> p k j", p=P))

                def evac_kb(m, ps):
                    for j in range(NJ):
                        nc.vector.tensor_copy(
                            kbT[m][:, j * 512:(j + 1) * 512], ps[j][:])
                proj_blocks(wuk_t, KC, cts, 0, NHG, evac_kb)

                # v natural: stationary = c token-block, moving = W_U_V k-tile
                for lt in range(LK):
                    ps = psA.tile([P, DQB], F32, name="ps_v", tag="ps")
                    for k in range(KC):
                        nc.tensor.matmul(
                            ps[:], cts[k][:, lt * P:(lt + 1) * P], wuv_t[:, k, :],
                            start=(k == 0), stop=(k == KC - 1))
                    nc.vector.tensor_copy(vts[lt][:], ps[:])

                # prefetch W_O for phase C while B runs
                wo_t = wop.tile([P, NHG, D], BF16, name="wo_t", tag="wo")
                nc.sync.dma_start(
                    out=wo_t[:], in_=wo.rearrange("(k p) j -> p k j", p=P))

              # ================= Phase B: attention =========================
              with tc.tile_pool(name="ptp", bufs=1) as ptp, \
                   tc.tile_pool(name="rcp", bufs=2) as rcp, \
                   tc.tile_pool(name="accp", bufs=1) as accp, \
                   tc.tile_pool(name="psB", bufs=1, space="PSUM") as psB:
                ones_f = ptp.tile([P, P], F32R, name="ones_f", tag="ones")
                nc.sync.dma_start(out=ones_f[:], in_=onesf[:, :])
                LAG = 2  # PV trails exp by LAG key-tiles so PE never waits ACT
                for h in range(NHG):
                    qr_m, ro = qrT[h // 2], (h % 2) * DHR
                    kr_m = krT[h // 2]
                    # pt buffered per lq-PAIR: [keys, lk, 1024] so exp runs
                    # on 1024-wide tiles (half the ACT per-op overhead)
                    ptb = [ptp.tile([P, LK, 1024], BF16, name=f"ptb{pr}",
                                    tag=f"ptb{pr}") for pr in range(2)]
                    # key-tile running sums of exp(scores): the softmax
                    # denominator collapses to ONE short matmul per pair
                    acc = [accp.tile([P, 1024], F32R, name=f"acc{pr}",
                                     tag=f"acc{pr}") for pr in range(2)]
                    ot = [psB.tile([P, 512], F32, name="ot", tag="ot", bufs=4)
                          for _ in range(NJ)]

                    def pv(j):
                        for lq in range(NJ):
                            nc.tensor.matmul(
                                ot[lq][:], vts[j][:, h * DH:(h + 1) * DH],
                                ptb[lq // 2][:, j,
                                             (lq % 2) * 512:(lq % 2) * 512 + 512],
                                start=(j == 0), stop=(j == LK - 1))

                    for lk in range(LK):
                        ksl = slice(lk * P, (lk + 1) * P)
                        sts = [psB.tile([P, 1024], F32, name="st", tag="st",
                                        bufs=2) for _ in range(2)]
                        for lq in range(NJ):   # kb stationary held for 4 mms
                            nc.tensor.matmul(
                                sts[lq // 2][:, (lq % 2) * 512:(lq % 2) * 512 + 512],
                                kbT[h][:, ksl],
                                qbT[h][:, lq * 512:(lq + 1) * 512],
                                start=True, stop=False)
                        for lq in range(NJ):   # kr stationary held for 4 mms
                            nc.tensor.matmul(
                                sts[lq // 2][:, (lq % 2) * 512:(lq % 2) * 512 + 512],
                                kr_m[ro:ro + DHR, ksl],
                                qr_m[ro:ro + DHR, lq * 512:(lq + 1) * 512],
                                start=False, stop=True)
                        for pr in range(2):
                            nc.scalar.activation(
                                ptb[pr][:, lk, :], sts[pr][:],
                                mybir.ActivationFunctionType.Exp, scale=SCALE)
                            with nc.allow_low_precision("f32 acc of bf16 pt"):
                                if lk == 0:
                                    nc.vector.tensor_copy(acc[pr][:],
                                                          ptb[pr][:, 0, :])
                                else:
                                    nc.vector.tensor_add(acc[pr][:], acc[pr][:],
                                                         ptb[pr][:, lk, :])
                        if lk >= LAG:
                            pv(lk - LAG)
                    for j in range(LK - LAG, LK):
                        pv(j)
                    for pr in range(2):
                        # softmax denominator: partition-sum the key-tile
                        # running sums with a single ones-matmul per half
                        rs = psB.tile([P, 1024], F32, name="rs", tag="st",
                                      bufs=2)
                        for half in range(2):
                            hsl = slice(half * 512, half * 512 + 512)
                            nc.tensor.matmul(rs[:, hsl], ones_f[:],
                                             acc[pr][:, hsl],
                                             start=True, stop=True)
                        rec = rcp.tile([P, 1024], F32, name="rec", tag="rec")
                        nc.vector.reciprocal_approx_fast(rec[:], rs[:])
                        for half in range(2):
                            lq = pr * 2 + half
                            nc.vector.tensor_mul(
                                oT[h][:, lq * 512:(lq + 1) * 512], ot[lq][:],
                                rec[:, half * 512:half * 512 + 512])

              # ================= Phase C: output projection =================
              with tc.tile_pool(name="ocp", bufs=4) as ocp, \
                   tc.tile_pool(name="psC", bufs=8, space="PSUM") as psC:
                for mt in range(L // P):
                    msl = slice(mt * P, (mt + 1) * P)
                    ps = [psC.tile([P, 512], F32, name="ps_o", tag="psc")
                          for _ in range(NJ)]
                    for k in range(NHG):
                        stat = oT[k][:, msl]
                        for nt in range(NJ):
                            nc.tensor.matmul(
                                ps[nt][:], stat, wo_t[:, k, nt * 512:(nt + 1) * 512],
                                start=(k == 0), stop=(k == NHG - 1))
                    oc = ocp.tile([P, D], BF16, name="oc", tag="oc")
                    for nt in range(NJ):
                        nc.vector.tensor_copy(oc[:, nt * 512:(nt + 1) * 512],
                                              ps[nt][:])
                    nc.sync.dma_start(out=out[msl, :], in_=oc[:])

    nc.compile()
    return nc


def _rope_tables():
    """cos/sin in transposed, 2-head-replicated layout (128 x L), plus Prot^T."""
    inv_freq = 1.0 / (ROPE_THETA ** (np.arange(0, DHR, 2, dtype=np.float32) / DHR))
    ang = np.arange(L, dtype=np.float32)[:, None] * inv_freq[None, :]  # (L, 32)
    cos64 = np.concatenate([np.cos(ang), np.cos(ang)], axis=1).T  # (64, L)
    sin64 = np.concatenate([np.sin(ang), np.sin(ang)], axis=1).T
    cosr = np.ascontiguousarray(np.tile(cos64, (2, 1)), dtype=np.float32)
    sinr = np.ascontiguousarray(np.tile(sin64, (2, 1)), dtype=np.float32)
    # rot(x) = [-x2, x1] per 64-dim head; block-diag over 2 heads; transposed.
    p64 = np.zeros((DHR, DHR), dtype=np.float32)
    half = DHR // 2
    p64[np.arange(half), np.arange(half) + half] = -1.0
    p64[np.arange(half) + half, np.arange(half)] = 1.0
    p128 = np.zeros((P, P), dtype=np.float32)
    p128[:DHR, :DHR] = p64
    p128[DHR:, DHR:] = p64
    protT = np.ascontiguousarray(p128.T)
    return cosr, sinr, protT


def _bf16():
    return mybir.dt.np(BF16)


def build_in_maps(x, W_D_Q, W_U_Q, W_Q_R, W_D_KV, W_U_K, W_K_R, W_U_V, W_O):
    cosr, sinr, protT = _rope_tables()
    bf = _bf16()
    f = np.float32
    x = np.asarray(x, f)
    W_D_Q = np.asarray(W_D_Q, f)
    # fold the low-rank query compression into single input->head projections
    Wq_full = (W_D_Q @ np.asarray(W_U_Q, f)).astype(bf)     # (D, NH*DH)
    Wqr_full = (W_D_Q @ np.asarray(W_Q_R, f)).astype(bf)    # (D, NH*DHR)
    xTs = [np.ascontiguousarray(x[b].T).astype(bf) for b in range(B)]
    wkv_b = np.ascontiguousarray(np.asarray(W_D_KV, f)).astype(bf)
    in_maps = []
    for c in range(8):
        b, g = c // 4, c % 4
        hb = slice(g * DQB, (g + 1) * DQB)
        hr = slice(g * DQR, (g + 1) * DQR)
        in_maps.append(dict(
            xT=xTs[b],
            wq=np.ascontiguousarray(Wq_full[:, hb]),
            wqr=np.ascontiguousarray(Wqr_full[:, hr]),
            wkv=wkv_b,
            wuk=np.ascontiguousarray(np.asarray(W_U_K, f)[:, hb]).astype(bf),
            wkr=np.ascontiguousarray(np.asarray(W_K_R, f)[:, hr]).astype(bf),
            wuv=np.ascontiguousarray(np.asarray(W_U_V, f)[:, hb]).astype(bf),
            wo=np.ascontiguousarray(np.asarray(W_O, f)[hb, :]).astype(bf),
            cosr=cosr, sinr=sinr,
            protT=protT.astype(bf),
            onesf=np.ones((P, P), dtype=np.float32),
        ))
    return in_maps


def kernel(x, W_D_Q, W_U_Q, W_Q_R, W_D_KV, W_U_K, W_K_R, W_U_V, W_O):
    if "nc" not in _CACHED:
        _CACHED["nc"] = _build()
    nc = _CACHED["nc"]
    in_maps = build_in_maps(x, W_D_Q, W_U_Q, W_Q_R, W_D_KV, W_U_K, W_K_R,
                            W_U_V, W_O)
    res = run_bass_kernel_spmd(nc, in_maps, core_ids=list(range(8)))
    outs = [r["out"].astype(np.float32) for r in res.results]
    full = np.stack(
        [outs[b * 4] + outs[b * 4 + 1] + outs[b * 4 + 2] + outs[b * 4 + 3]
         for b in range(B)]).astype(np.float32)
    return full
